# revision 1
# baseline (speedup 1.0000x reference)
"""DEMA (double exponential smoothing) Trainium2 kernel.

x: [64, 2048, 512] fp32; recurrence over T=2048 is a 2x2 linear
time-invariant system per (batch, channel) lane:

    z_t = A z_{t-1} + B x_t,   y_t = e1^T z_t
    A = [[1-a, 1-a], [-ab, 1-ab]],  B = [a, ab]^T

Blocked scan: chunks of L=126 timesteps. One [128x128] @ [128x512]
fp32 matmul per (batch, chunk): rhs rows 0-1 carry the (s, b) state
into the chunk, rows 2..127 carry the chunk's inputs; lhsT columns
0-1 produce the chunk-end state (fed into the next chunk's rhs rows
0-1 via a tiny PSUM->SBUF copy), columns 2..127 produce the outputs.
Batch dim is sharded 8 ways across cores; within a core the 8
batches' chunk chains are interleaved chunk-major so the PE always
has independent work while each carry chain advances.
"""

import sys

import numpy as np

if "/opt/trn_rl_repo" not in sys.path:
    sys.path.insert(0, "/opt/trn_rl_repo")

B, T, C = 64, 2048, 512
NCORES = 8
BPC = B // NCORES  # batches per core
L = 126            # timesteps per full chunk (126 outputs + 2 state cols = 128)
NFULL = 16         # full chunks cover t = 0..2015
LT = T - NFULL * L  # tail chunk, 32 timesteps

_cache = {}


def _build_mats(alpha, beta):
    """Per-call host precompute of the chunk transfer matrices (float64)."""
    a = np.float64(alpha)
    b = np.float64(beta)
    A = np.array([[1 - a, 1 - a], [-a * b, 1 - a * b]], dtype=np.float64)
    Bv = np.array([a, a * b], dtype=np.float64)
    Ap = [np.eye(2)]
    for _ in range(L):
        Ap.append(Ap[-1] @ A)
    AB = np.stack([Ap[j] @ Bv for j in range(L)])  # [L, 2], A^j B
    w = AB[:, 0]                                   # w_j = e1^T A^j B

    # Generic chunk starting at t0, carry z_{t0-1} in rhs rows 0-1:
    #   z_{t0+tau} = A^{tau+1} z_{t0-1} + sum_k A^{tau-k} B x_{t0+k}
    G1 = np.zeros((128, 128))
    for tau in range(L):
        m = 2 + tau
        G1[0, m] = Ap[tau + 1][0, 0]
        G1[1, m] = Ap[tau + 1][0, 1]
        for k in range(tau + 1):
            G1[2 + k, m] = w[tau - k]
    for j in range(2):
        for jp in range(2):
            G1[j, jp] = Ap[L][jp, j]
    for k in range(L):
        G1[2 + k, 0] = AB[L - 1 - k][0]
        G1[2 + k, 1] = AB[L - 1 - k][1]

    # Chunk 0: z_0 = (x_0, x_1 - x_0), y_0 = x_0, rhs rows 0-1 are zero.
    G0 = np.zeros((128, 128))
    G0[2, 2] = 1.0
    for tau in range(1, L):
        m = 2 + tau
        G0[2, m] = Ap[tau][0, 0] - Ap[tau][0, 1]
        G0[3, m] = Ap[tau][0, 1] + w[tau - 1]
        for k in range(2, tau + 1):
            G0[2 + k, m] = w[tau - k]
    for jp in range(2):
        G0[2, jp] = Ap[L - 1][jp, 0] - Ap[L - 1][jp, 1]
        G0[3, jp] = Ap[L - 1][jp, 1] + AB[L - 2][jp]
        for k in range(2, L):
            G0[2 + k, jp] = AB[L - 1 - k][jp]

    # Tail chunk: LT outputs, no state columns.
    Gt = np.zeros((2 + LT, LT))
    for tau in range(LT):
        Gt[0, tau] = Ap[tau + 1][0, 0]
        Gt[1, tau] = Ap[tau + 1][0, 1]
        for k in range(tau + 1):
            Gt[2 + k, tau] = w[tau - k]
    return G0.astype(np.float32), G1.astype(np.float32), Gt.astype(np.float32)


def _build_program():
    import concourse.mybir as mybir
    import concourse.tile as tile
    from concourse import bacc

    FP32 = mybir.dt.float32
    nc = bacc.Bacc(
        "TRN2", target_bir_lowering=False, debug=False, enable_asserts=False
    )
    x_d = nc.dram_tensor("x", [BPC, T, C], FP32, kind="ExternalInput").ap()
    g0_d = nc.dram_tensor("g0", [128, 128], FP32, kind="ExternalInput").ap()
    g1_d = nc.dram_tensor("g1", [128, 128], FP32, kind="ExternalInput").ap()
    gt_d = nc.dram_tensor("gt", [2 + LT, LT], FP32, kind="ExternalInput").ap()
    y_d = nc.dram_tensor("y", [BPC, T, C], FP32, kind="ExternalOutput").ap()

    with tile.TileContext(nc) as tc:
        with (
            tc.tile_pool(name="g", bufs=1) as gpool,
            tc.tile_pool(name="xp", bufs=18) as xpool,
            tc.tile_pool(name="op", bufs=12) as opool,
            tc.tile_pool(name="ps", bufs=6, space="PSUM") as pspool,
        ):
            g0 = gpool.tile([128, 128], FP32, tag="g0")
            g1 = gpool.tile([128, 128], FP32, tag="g1")
            gt = gpool.tile([2 + LT, LT], FP32, tag="gt")
            nc.sync.dma_start(out=g0[:], in_=g0_d)
            nc.sync.dma_start(out=g1[:], in_=g1_d)
            nc.sync.dma_start(out=gt[:], in_=gt_d)

            xcur = []
            for b in range(BPC):
                t0 = xpool.tile([128, C], FP32, tag="x")
                nc.gpsimd.memset(t0[0:2, :], 0.0)
                nc.sync.dma_start(out=t0[2:128, :], in_=x_d[b, 0:L, :])
                xcur.append(t0)

            ncopy = 0
            for i in range(NFULL + 1):
                for b in range(BPC):
                    cur = xcur[b]
                    if i < NFULL:
                        ps = pspool.tile([128, C], FP32, tag="ps")
                        nc.tensor.matmul(
                            ps[:], (g0 if i == 0 else g1)[:], cur[:],
                            start=True, stop=True,
                        )
                        if i + 1 < NFULL:
                            nxt = xpool.tile([128, C], FP32, tag="x")
                            nc.sync.dma_start(
                                out=nxt[2:128, :],
                                in_=x_d[b, L * (i + 1):L * (i + 2), :],
                            )
                        else:
                            nxt = xpool.tile([2 + LT, C], FP32, tag="x")
                            nc.sync.dma_start(
                                out=nxt[2:2 + LT, :], in_=x_d[b, L * NFULL:T, :]
                            )
                        nc.vector.tensor_copy(out=nxt[0:2, :], in_=ps[0:2, :])
                        xcur[b] = nxt
                        o = opool.tile([128, C], FP32, tag="o")
                        if ncopy % 2 == 0:
                            nc.scalar.copy(out=o[:], in_=ps[:])
                        else:
                            nc.vector.tensor_copy(out=o[:], in_=ps[:])
                        ncopy += 1
                        nc.sync.dma_start(
                            out=y_d[b, L * i:L * (i + 1), :], in_=o[2:128, :]
                        )
                    else:
                        ps = pspool.tile([128, C], FP32, tag="ps")
                        nc.tensor.matmul(
                            ps[0:LT, :], gt[:], cur[0:2 + LT, :],
                            start=True, stop=True,
                        )
                        o = opool.tile([LT, C], FP32, tag="o")
                        if ncopy % 2 == 0:
                            nc.scalar.copy(out=o[:], in_=ps[0:LT, :])
                        else:
                            nc.vector.tensor_copy(out=o[:], in_=ps[0:LT, :])
                        ncopy += 1
                        nc.sync.dma_start(out=y_d[b, L * NFULL:T, :], in_=o[:])
    nc.compile()
    return nc


def _get_program():
    if "nc" not in _cache:
        _cache["nc"] = _build_program()
    return _cache["nc"]


def _run(x, alpha, beta, trace=False):
    from concourse.bass_utils import run_bass_kernel_spmd

    x = np.ascontiguousarray(np.asarray(x, dtype=np.float32))
    G0, G1, Gt = _build_mats(alpha, beta)
    nc = _get_program()
    in_maps = [
        {"x": x[c * BPC:(c + 1) * BPC], "g0": G0, "g1": G1, "gt": Gt}
        for c in range(NCORES)
    ]
    res = run_bass_kernel_spmd(nc, in_maps, list(range(NCORES)), trace=trace)
    out = np.concatenate([res.results[c]["y"] for c in range(NCORES)], axis=0)
    return out, res


def kernel(**inputs):
    alpha = float(np.asarray(inputs["alpha"]))
    beta = float(np.asarray(inputs["beta"]))
    out, _ = _run(inputs["x"], alpha, beta, trace=False)
    return out


# revision 2
# speedup vs baseline: 1.1130x; 1.1130x over previous
"""DEMA (double exponential smoothing) Trainium2 kernel.

x: [64, 2048, 512] fp32; recurrence over T=2048 is a 2x2 linear
time-invariant system per (batch, channel) lane:

    z_t = A z_{t-1} + B x_t,   y_t = e1^T z_t
    A = [[1-a, 1-a], [-ab, 1-ab]],  B = [a, ab]^T

Blocked scan: chunks of L=126 timesteps. One [128x128] @ [128x512]
fp32 matmul per (batch, chunk): rhs rows 0-1 carry the (s, b) state
into the chunk, rows 2..127 carry the chunk's inputs; lhsT columns
0-1 produce the chunk-end state (fed into the next chunk's rhs rows
0-1 via a tiny PSUM->SBUF copy), columns 2..127 produce the outputs.
Batch dim is sharded 8 ways across cores; within a core the 8
batches' chunk chains are interleaved chunk-major so the PE always
has independent work while each carry chain advances.
"""

import sys

import numpy as np

if "/opt/trn_rl_repo" not in sys.path:
    sys.path.insert(0, "/opt/trn_rl_repo")

B, T, C = 64, 2048, 512
NCORES = 8
BPC = B // NCORES  # batches per core
L = 126            # timesteps per full chunk (126 outputs + 2 state cols = 128)
NFULL = 16         # full chunks cover t = 0..2015
LT = T - NFULL * L  # tail chunk, 32 timesteps

_cache = {}


def _build_mats(alpha, beta):
    """Per-call host precompute of the chunk transfer matrices (float64)."""
    a = np.float64(alpha)
    b = np.float64(beta)
    A = np.array([[1 - a, 1 - a], [-a * b, 1 - a * b]], dtype=np.float64)
    Bv = np.array([a, a * b], dtype=np.float64)
    Ap = [np.eye(2)]
    for _ in range(L):
        Ap.append(Ap[-1] @ A)
    AB = np.stack([Ap[j] @ Bv for j in range(L)])  # [L, 2], A^j B
    w = AB[:, 0]                                   # w_j = e1^T A^j B

    # Generic chunk starting at t0, carry z_{t0-1} in rhs rows 0-1:
    #   z_{t0+tau} = A^{tau+1} z_{t0-1} + sum_k A^{tau-k} B x_{t0+k}
    G1 = np.zeros((128, 128))
    for tau in range(L):
        m = 2 + tau
        G1[0, m] = Ap[tau + 1][0, 0]
        G1[1, m] = Ap[tau + 1][0, 1]
        for k in range(tau + 1):
            G1[2 + k, m] = w[tau - k]
    for j in range(2):
        for jp in range(2):
            G1[j, jp] = Ap[L][jp, j]
    for k in range(L):
        G1[2 + k, 0] = AB[L - 1 - k][0]
        G1[2 + k, 1] = AB[L - 1 - k][1]

    # Chunk 0: z_0 = (x_0, x_1 - x_0), y_0 = x_0, rhs rows 0-1 are zero.
    G0 = np.zeros((128, 128))
    G0[2, 2] = 1.0
    for tau in range(1, L):
        m = 2 + tau
        G0[2, m] = Ap[tau][0, 0] - Ap[tau][0, 1]
        G0[3, m] = Ap[tau][0, 1] + w[tau - 1]
        for k in range(2, tau + 1):
            G0[2 + k, m] = w[tau - k]
    for jp in range(2):
        G0[2, jp] = Ap[L - 1][jp, 0] - Ap[L - 1][jp, 1]
        G0[3, jp] = Ap[L - 1][jp, 1] + AB[L - 2][jp]
        for k in range(2, L):
            G0[2 + k, jp] = AB[L - 1 - k][jp]

    # Tail chunk: LT outputs, no state columns.
    Gt = np.zeros((2 + LT, LT))
    for tau in range(LT):
        Gt[0, tau] = Ap[tau + 1][0, 0]
        Gt[1, tau] = Ap[tau + 1][0, 1]
        for k in range(tau + 1):
            Gt[2 + k, tau] = w[tau - k]
    return G0.astype(np.float32), G1.astype(np.float32), Gt.astype(np.float32)


NG = 2             # batch groups per core
GB = BPC // NG     # batches per group (4) -> group tiles are [*, GB*C]


def _build_program():
    import concourse.mybir as mybir
    import concourse.tile as tile
    from concourse import bacc

    FP32 = mybir.dt.float32
    W = GB * C  # group tile width in the free dim
    nc = bacc.Bacc(
        "TRN2", target_bir_lowering=False, debug=False, enable_asserts=False
    )
    x_d = nc.dram_tensor("x", [BPC, T, C], FP32, kind="ExternalInput").ap()
    g0_d = nc.dram_tensor("g0", [128, 128], FP32, kind="ExternalInput").ap()
    g1_d = nc.dram_tensor("g1", [128, 128], FP32, kind="ExternalInput").ap()
    gt_d = nc.dram_tensor("gt", [2 + LT, LT], FP32, kind="ExternalInput").ap()
    y_d = nc.dram_tensor("y", [BPC, T, C], FP32, kind="ExternalOutput").ap()

    with tile.TileContext(nc) as tc:
        with (
            tc.tile_pool(name="g", bufs=1) as gpool,
            tc.tile_pool(name="xp", bufs=6) as xpool,
            tc.tile_pool(name="op", bufs=5) as opool,
            tc.tile_pool(name="ps", bufs=2, space="PSUM") as pspool,
        ):
            g0 = gpool.tile([128, 128], FP32, tag="g0")
            g1 = gpool.tile([128, 128], FP32, tag="g1")
            gt = gpool.tile([2 + LT, LT], FP32, tag="gt")
            nc.sync.dma_start(out=g0[:], in_=g0_d)
            nc.sync.dma_start(out=g1[:], in_=g1_d)
            nc.sync.dma_start(out=gt[:], in_=gt_d)

            xcur = []
            for g in range(NG):
                xs = xpool.tile([128, W], FP32, tag="x")
                nc.gpsimd.memset(xs[0:2, :], 0.0)
                for bb in range(GB):
                    b = g * GB + bb
                    nc.sync.dma_start(
                        out=xs[2:128, bb * C:(bb + 1) * C], in_=x_d[b, 0:L, :]
                    )
                xcur.append(xs)

            ncopy = 0
            for i in range(NFULL + 1):
                for g in range(NG):
                    xs = xcur[g]
                    ps = pspool.tile([128, W], FP32, tag="ps")
                    if i < NFULL:
                        gmat = g0 if i == 0 else g1
                        for bb in range(GB):
                            nc.tensor.matmul(
                                ps[:, bb * C:(bb + 1) * C], gmat[:],
                                xs[:, bb * C:(bb + 1) * C],
                                start=True, stop=True,
                            )
                        # prefetch next round's inputs for this group
                        if i + 1 < NFULL:
                            nxt = xpool.tile([128, W], FP32, tag="x")
                            for bb in range(GB):
                                b = g * GB + bb
                                nc.sync.dma_start(
                                    out=nxt[2:128, bb * C:(bb + 1) * C],
                                    in_=x_d[b, L * (i + 1):L * (i + 2), :],
                                )
                        else:
                            nxt = xpool.tile([2 + LT, W], FP32, tag="x")
                            for bb in range(GB):
                                b = g * GB + bb
                                nc.sync.dma_start(
                                    out=nxt[2:2 + LT, bb * C:(bb + 1) * C],
                                    in_=x_d[b, L * NFULL:T, :],
                                )
                        # carry: chunk-end states -> next rhs rows 0-1
                        nc.vector.tensor_copy(out=nxt[0:2, :], in_=ps[0:2, :])
                        xcur[g] = nxt
                        o = opool.tile([128, W], FP32, tag="o")
                        if ncopy % 2 == 0:
                            nc.scalar.copy(out=o[:], in_=ps[:])
                        else:
                            nc.vector.tensor_copy(out=o[:], in_=ps[:])
                        ncopy += 1
                        for bb in range(GB):
                            b = g * GB + bb
                            eng = nc.scalar if bb % 2 == 0 else nc.gpsimd
                            eng.dma_start(
                                out=y_d[b, L * i:L * (i + 1), :],
                                in_=o[2:128, bb * C:(bb + 1) * C],
                            )
                    else:  # tail chunk (32 steps, no state outputs)
                        for bb in range(GB):
                            nc.tensor.matmul(
                                ps[0:LT, bb * C:(bb + 1) * C], gt[:],
                                xs[0:2 + LT, bb * C:(bb + 1) * C],
                                start=True, stop=True,
                            )
                        o = opool.tile([LT, W], FP32, tag="o")
                        if ncopy % 2 == 0:
                            nc.scalar.copy(out=o[:], in_=ps[0:LT, :])
                        else:
                            nc.vector.tensor_copy(out=o[:], in_=ps[0:LT, :])
                        ncopy += 1
                        for bb in range(GB):
                            b = g * GB + bb
                            eng = nc.scalar if bb % 2 == 0 else nc.gpsimd
                            eng.dma_start(
                                out=y_d[b, L * NFULL:T, :],
                                in_=o[:, bb * C:(bb + 1) * C],
                            )
    nc.compile()
    return nc


def _get_program():
    if "nc" not in _cache:
        _cache["nc"] = _build_program()
    return _cache["nc"]


def _run(x, alpha, beta, trace=False):
    from concourse.bass_utils import run_bass_kernel_spmd

    x = np.ascontiguousarray(np.asarray(x, dtype=np.float32))
    G0, G1, Gt = _build_mats(alpha, beta)
    nc = _get_program()
    in_maps = [
        {"x": x[c * BPC:(c + 1) * BPC], "g0": G0, "g1": G1, "gt": Gt}
        for c in range(NCORES)
    ]
    res = run_bass_kernel_spmd(nc, in_maps, list(range(NCORES)), trace=trace)
    out = np.concatenate([res.results[c]["y"] for c in range(NCORES)], axis=0)
    return out, res


def kernel(**inputs):
    alpha = float(np.asarray(inputs["alpha"]))
    beta = float(np.asarray(inputs["beta"]))
    out, _ = _run(inputs["x"], alpha, beta, trace=False)
    return out


# revision 5
# speedup vs baseline: 1.2657x; 1.1372x over previous
"""DEMA (double exponential smoothing) Trainium2 kernel.

x: [64, 2048, 512] fp32; recurrence over T=2048 is a 2x2 linear
time-invariant system per (batch, channel) lane:

    z_t = A z_{t-1} + B x_t,   y_t = e1^T z_t
    A = [[1-a, 1-a], [-ab, 1-ab]],  B = [a, ab]^T

Blocked scan: chunks of L=126 timesteps. One [128x128] @ [128x512]
fp32 matmul per (batch, chunk): rhs rows 0-1 carry the (s, b) state
into the chunk, rows 2..127 carry the chunk's inputs; lhsT columns
0-1 produce the chunk-end state (fed into the next chunk's rhs rows
0-1 via a tiny PSUM->SBUF copy), columns 2..127 produce the outputs.
Batch dim is sharded 8 ways across cores; within a core the 8
batches' chunk chains are interleaved chunk-major so the PE always
has independent work while each carry chain advances.
"""

import sys

import numpy as np

if "/opt/trn_rl_repo" not in sys.path:
    sys.path.insert(0, "/opt/trn_rl_repo")

B, T, C = 64, 2048, 512
NCORES = 8
BPC = B // NCORES  # batches per core
L = 126            # timesteps per full chunk (126 outputs + 2 state cols = 128)
NFULL = 16         # full chunks cover t = 0..2015
LT = T - NFULL * L  # tail chunk, 32 timesteps

_cache = {}


def _build_mats(alpha, beta):
    """Per-call host precompute of the chunk transfer matrices (float64)."""
    a = np.float64(alpha)
    b = np.float64(beta)
    A = np.array([[1 - a, 1 - a], [-a * b, 1 - a * b]], dtype=np.float64)
    Bv = np.array([a, a * b], dtype=np.float64)
    Ap = [np.eye(2)]
    for _ in range(L):
        Ap.append(Ap[-1] @ A)
    AB = np.stack([Ap[j] @ Bv for j in range(L)])  # [L, 2], A^j B
    w = AB[:, 0]                                   # w_j = e1^T A^j B

    # Generic chunk starting at t0, carry z_{t0-1} in rhs rows 0-1:
    #   z_{t0+tau} = A^{tau+1} z_{t0-1} + sum_k A^{tau-k} B x_{t0+k}
    G1 = np.zeros((128, 128))
    for tau in range(L):
        m = 2 + tau
        G1[0, m] = Ap[tau + 1][0, 0]
        G1[1, m] = Ap[tau + 1][0, 1]
        for k in range(tau + 1):
            G1[2 + k, m] = w[tau - k]
    for j in range(2):
        for jp in range(2):
            G1[j, jp] = Ap[L][jp, j]
    for k in range(L):
        G1[2 + k, 0] = AB[L - 1 - k][0]
        G1[2 + k, 1] = AB[L - 1 - k][1]

    # Chunk 0: z_0 = (x_0, x_1 - x_0), y_0 = x_0, rhs rows 0-1 are zero.
    G0 = np.zeros((128, 128))
    G0[2, 2] = 1.0
    for tau in range(1, L):
        m = 2 + tau
        G0[2, m] = Ap[tau][0, 0] - Ap[tau][0, 1]
        G0[3, m] = Ap[tau][0, 1] + w[tau - 1]
        for k in range(2, tau + 1):
            G0[2 + k, m] = w[tau - k]
    for jp in range(2):
        G0[2, jp] = Ap[L - 1][jp, 0] - Ap[L - 1][jp, 1]
        G0[3, jp] = Ap[L - 1][jp, 1] + AB[L - 2][jp]
        for k in range(2, L):
            G0[2 + k, jp] = AB[L - 1 - k][jp]

    # Tail chunk: LT outputs, no state columns.
    Gt = np.zeros((2 + LT, LT))
    for tau in range(LT):
        Gt[0, tau] = Ap[tau + 1][0, 0]
        Gt[1, tau] = Ap[tau + 1][0, 1]
        for k in range(tau + 1):
            Gt[2 + k, tau] = w[tau - k]
    return G0.astype(np.float32), G1.astype(np.float32), Gt.astype(np.float32)


NG = 4             # batch groups per core
GB = BPC // NG     # batches per group (2) -> group tiles are [*, GB*C]


def _build_program():
    import concourse.mybir as mybir
    import concourse.tile as tile
    from concourse import bacc

    FP32 = mybir.dt.float32
    W = GB * C  # group tile width in the free dim
    nc = bacc.Bacc(
        "TRN2", target_bir_lowering=False, debug=False, enable_asserts=False
    )
    x_d = nc.dram_tensor("x", [BPC, T, C], FP32, kind="ExternalInput").ap()
    g0_d = nc.dram_tensor("g0", [128, 128], FP32, kind="ExternalInput").ap()
    g1_d = nc.dram_tensor("g1", [128, 128], FP32, kind="ExternalInput").ap()
    gt_d = nc.dram_tensor("gt", [2 + LT, LT], FP32, kind="ExternalInput").ap()
    y_d = nc.dram_tensor("y", [BPC, T, C], FP32, kind="ExternalOutput").ap()

    with tile.TileContext(nc) as tc:
        with (
            tc.tile_pool(name="g", bufs=1) as gpool,
            tc.tile_pool(name="xp", bufs=10) as xpool,
            tc.tile_pool(name="op", bufs=8) as opool,
            tc.tile_pool(name="ps", bufs=4, space="PSUM") as pspool,
        ):
            g0 = gpool.tile([128, 128], FP32, tag="g0")
            g1 = gpool.tile([128, 128], FP32, tag="g1")
            gt = gpool.tile([2 + LT, LT], FP32, tag="gt")
            nc.sync.dma_start(out=g0[:], in_=g0_d)
            nc.sync.dma_start(out=g1[:], in_=g1_d)
            nc.sync.dma_start(out=gt[:], in_=gt_d)

            xcur = []
            for g in range(NG):
                xs = xpool.tile([128, W], FP32, tag="x")
                nc.gpsimd.memset(xs[0:2, :], 0.0)
                for bb in range(GB):
                    b = g * GB + bb
                    nc.sync.dma_start(
                        out=xs[2:128, bb * C:(bb + 1) * C], in_=x_d[b, 0:L, :]
                    )
                xcur.append(xs)

            ncopy = 0
            for i in range(NFULL + 1):
                for g in range(NG):
                    xs = xcur[g]
                    ps = pspool.tile([128, W], FP32, tag="ps")
                    if i < NFULL:
                        gmat = g0 if i == 0 else g1
                        # next round's input tile for this group
                        if i + 1 < NFULL:
                            nxt = xpool.tile([128, W], FP32, tag="x")
                            nrows = L
                        else:
                            nxt = xpool.tile([2 + LT, W], FP32, tag="x")
                            nrows = LT
                        for bb in range(GB):
                            b = g * GB + bb
                            sl = slice(bb * C, (bb + 1) * C)
                            nc.tensor.matmul(
                                ps[:, sl], gmat[:], xs[:, sl],
                                start=True, stop=True,
                            )
                            # per-batch carry into next rhs rows 0-1 (keeps
                            # the chunk chain latency at one small DVE op)
                            nc.vector.tensor_copy(
                                out=nxt[0:2, sl], in_=ps[0:2, sl]
                            )
                            nc.sync.dma_start(
                                out=nxt[2:2 + nrows, sl],
                                in_=x_d[b, L * (i + 1):L * (i + 1) + nrows, :],
                            )
                        xcur[g] = nxt
                        o = opool.tile([128, W], FP32, tag="o")
                        nc.scalar.copy(out=o[:], in_=ps[:])
                        for bb in range(GB):
                            b = g * GB + bb
                            nc.gpsimd.dma_start(
                                out=y_d[b, L * i:L * (i + 1), :],
                                in_=o[2:128, bb * C:(bb + 1) * C],
                            )
                    else:  # tail chunk (32 steps, no state outputs)
                        for bb in range(GB):
                            sl = slice(bb * C, (bb + 1) * C)
                            nc.tensor.matmul(
                                ps[0:LT, sl], gt[:], xs[0:2 + LT, sl],
                                start=True, stop=True,
                            )
                        o = opool.tile([LT, W], FP32, tag="o")
                        nc.scalar.copy(out=o[:], in_=ps[0:LT, :])
                        for bb in range(GB):
                            b = g * GB + bb
                            nc.gpsimd.dma_start(
                                out=y_d[b, L * NFULL:T, :],
                                in_=o[:, bb * C:(bb + 1) * C],
                            )
    nc.compile()
    return nc


def _get_program():
    if "nc" not in _cache:
        _cache["nc"] = _build_program()
    return _cache["nc"]


def _run(x, alpha, beta, trace=False):
    from concourse.bass_utils import run_bass_kernel_spmd

    x = np.ascontiguousarray(np.asarray(x, dtype=np.float32))
    G0, G1, Gt = _build_mats(alpha, beta)
    nc = _get_program()
    in_maps = [
        {"x": x[c * BPC:(c + 1) * BPC], "g0": G0, "g1": G1, "gt": Gt}
        for c in range(NCORES)
    ]
    res = run_bass_kernel_spmd(nc, in_maps, list(range(NCORES)), trace=trace)
    out = np.concatenate([res.results[c]["y"] for c in range(NCORES)], axis=0)
    return out, res


def kernel(**inputs):
    alpha = float(np.asarray(inputs["alpha"]))
    beta = float(np.asarray(inputs["beta"]))
    out, _ = _run(inputs["x"], alpha, beta, trace=False)
    return out
